# revision 17
# baseline (speedup 1.0000x reference)
"""Trainium2 Bass kernel for nn_CoAttention_TextDNS.

Math: both additive co-attention blocks have scores of the form
    score[l, m] = f(l) + g(m) + const
followed by softmax over the last axis, so the row-dependent terms cancel
(softmax shift invariance) and the attention weights are identical for every
row l:
    att_dns[b]  = broadcast_rows( softmax(tanh(dns[b]  @ W_d1.T) @ wb) @ dns[b] )
    att_text[b] = broadcast_rows( softmax(tanh(text[b] @ W_t2.T) @ wd) @ text[b] )
with wb = w_att1[H:], wd = w_att2[H:].  W_t1/b_t1/W_d2/b_d2/wa/wc/b_att1/
b_att2 do not affect the output.

Sharding: data-parallel over batch, one batch element per NeuronCore (B=8).

Device schedule (everything bf16 on the wires, f32 accumulation):
 - The GEMMs are computed TRANSPOSED (out[o, l] = sum_h W[o,h] x[l,h]) in
   128-row o-tiles, so the tanh@w projection is a tiny accumulating PE
   matmul producing the score column [l, 1] directly: no transposes, no
   w-broadcast, no DVE work at all.
 - softmax: exp on ACT; the kernel ships the UNNORMALIZED weighted row-sum
   v_raw = X.T @ e (PE matmuls against natural-layout activations) plus an
   all-partitions broadcast of sum(e) (a ones-matrix matmul) in one
   PSUM-source DMA; the host divides.  This removes the whole
   copy->broadcast->reciprocal->scale tail chain.
 - DMA triggers are spread across Pool/SP/ACT and ordered so the
   accumulating GEMMs never wait on a late chunk; the d1 path consumes
   o-tiles in stream-arrival order [0,1,4,5,2,3].
 - PE p-state is warmed with junk matmuls sized to end right as the first
   operands land.
"""

import numpy as np
import ml_dtypes

B, L, M, H = 8, 256, 128, 768
HC = H // 128  # 6 contraction / output chunks of 128
NWARM = 13


def _build_module(reps=1):
    """Build the per-core module. reps>1 wraps the pipeline in an on-device
    hardware loop -- used only for wall-clock benchmarking (the ~70 ms axon
    dispatch RTT swamps a single ~15 us execution)."""
    import concourse.tile as tile
    from concourse import bacc, mybir
    from contextlib import nullcontext

    f32 = mybir.dt.float32
    bf16 = mybir.dt.bfloat16

    nc = bacc.Bacc("TRN2", target_bir_lowering=False, debug=False)

    # dnsT fused with the tiny projection-weight columns (cols HC*M..)
    dnsT = nc.dram_tensor("dnsT", [128, HC * M + 2 * HC], bf16, kind="ExternalInput").ap()
    textT = nc.dram_tensor("textT", [128, HC * L], bf16, kind="ExternalInput").ap()
    dnsN = nc.dram_tensor("dnsN", [128, H], bf16, kind="ExternalInput").ap()
    textN = nc.dram_tensor("textN", [128, 2 * H], bf16, kind="ExternalInput").ap()
    wd1f = nc.dram_tensor("wd1f", [128, HC * HC * 128], bf16, kind="ExternalInput").ap()
    wt2f = nc.dram_tensor("wt2f", [128, HC * HC * 128], bf16, kind="ExternalInput").ap()
    # col 0: sum(e) broadcast; cols 1..6: unnormalized v chunks
    v1_out = nc.dram_tensor("v1", [128, 1 + HC], f32, kind="ExternalOutput").ap()
    v2_out = nc.dram_tensor("v2", [128, 1 + HC], f32, kind="ExternalOutput").ap()

    Tanh = mybir.ActivationFunctionType.Tanh
    Exp = mybir.ActivationFunctionType.Exp

    with tile.TileContext(nc) as tc:
        with (
            tc.tile_pool(name="ins", bufs=1) as ins,
            tc.tile_pool(name="work", bufs=1) as work,
            tc.tile_pool(name="tanhp", bufs=3) as tanhp,
            tc.tile_pool(name="gemm", bufs=2, space="PSUM") as gemm,
            tc.tile_pool(name="small", bufs=1, space="PSUM") as small,
        ):
            # One 1.0-memset serves as PE-warm operand, ones-matrix (for the
            # sum broadcast) and ones-column.
            ones = ins.tile([128, 128], bf16, tag="ones")
            nc.vector.memset(ones, 1.0)

            # Warm the PE p-state with junk matmuls so the real GEMMs run at
            # full clock; sized to end just after the first operands land.
            warm_ps = gemm.tile([128, L], f32, tag="g2")
            for _ in range(NWARM):
                nc.tensor.matmul(
                    warm_ps[:, 0:128], ones, ones, start=True, stop=True
                )

            loop = tc.For_i(0, reps, 1) if reps > 1 else nullcontext()
            with loop:
                _pipeline_body(nc, tc, ins, work, tanhp, gemm, small, mybir,
                               dnsT, textT, dnsN, textN, wd1f, wt2f,
                               v1_out, v2_out, ones, Tanh, Exp, f32, bf16)

    nc.compile()
    return nc


def _pipeline_body(nc, tc, ins, work, tanhp, gemm, small, mybir,
                   dnsT, textT, dnsN, textN, wd1f, wt2f,
                   v1_out, v2_out, ones, Tanh, Exp, f32, bf16):
    from concourse.tile import add_dep_helper

    ones_mat = ones
    prev_mm = [None]

    def pe_mm(*args, **kwargs):
        mm = nc.tensor.matmul(*args, **kwargs)
        if prev_mm[0] is not None:
            add_dep_helper(mm.ins, prev_mm[0].ins, sync=False,
                           reason="keep PE queue in stream order")
        prev_mm[0] = mm
        return mm

    # ---- DMAs: spread across SP/Pool (ACT only pays the auto table load
    # and is then free for the tanh/exp stream) ---------------------------
    wd1_sb = ins.tile([128, HC, HC, 128], bf16, tag="wd1")
    wd1_r = wd1f.rearrange("p (t c o) -> p t c o", t=HC, c=HC)
    wt2_sb = ins.tile([128, HC, HC, 128], bf16, tag="wt2")
    wt2_r = wt2f.rearrange("p (t c o) -> p t c o", t=HC, c=HC)

    # SP: dnsT(+wcols), wd1f t2-t3, textT, natural-layout activations
    dnsTw_sb = ins.tile([128, HC * M + 2 * HC], bf16, tag="dnsTw")
    nc.sync.dma_start(out=dnsTw_sb, in_=dnsT)
    dnsT_sb = dnsTw_sb[:, 0 : HC * M].rearrange("p (c m) -> p c m", c=HC)
    wcols_sb = dnsTw_sb[:, HC * M : HC * M + 2 * HC]
    for t in (2, 3):
        nc.sync.dma_start(out=wd1_sb[:, t : t + 1], in_=wd1_r[:, t : t + 1])
    textT_sb = ins.tile([128, HC, L], bf16, tag="textT")
    nc.sync.dma_start(out=textT_sb, in_=textT.rearrange("p (c l) -> p c l", c=HC))
    dnsN_sb = ins.tile([128, H], bf16, tag="dnsN")
    nc.sync.dma_start(out=dnsN_sb, in_=dnsN)
    textN_sb = ins.tile([128, 2, H], bf16, tag="textN")
    nc.sync.dma_start(out=textN_sb, in_=textN.rearrange("p (j h) -> p j h", j=2))

    # Pool: wd1f t0,t1, then wt2f interleaved with wd1f t4 so the t2 GEMM's
    # first weight tile completes before the d1 phase drains
    for t in (0, 1):
        nc.gpsimd.dma_start(out=wd1_sb[:, t : t + 1], in_=wd1_r[:, t : t + 1])
    nc.gpsimd.dma_start(out=wt2_sb[:, 0:1], in_=wt2_r[:, 0:1])
    nc.gpsimd.dma_start(out=wd1_sb[:, 4:5], in_=wd1_r[:, 4:5])
    for t in range(1, HC):
        nc.gpsimd.dma_start(out=wt2_sb[:, t : t + 1], in_=wt2_r[:, t : t + 1])

    # ACT: wd1f t5 rides the idle window after the table load
    nc.scalar.dma_start(out=wd1_sb[:, 5:6], in_=wd1_r[:, 5:6])

    # ---- PSUM plan (bank-granular; one open accumulation group per bank):
    # g1 x2 | g2 x2 | u_ps (u1c,u2c0 sequential) | u2_ps (u2c1) | sv_ps
    u_ps = small.tile([128, 2], f32, tag="ucols")
    u2_ps = small.tile([128, 1], f32, tag="u2col")
    sv_ps = small.tile([128, 2 + 2 * HC], f32, tag="svcols")
    u1c = u_ps[:, 0:1]
    u2c = (u_ps[:, 1:2], u2_ps[:, 0:1])

    # ---- d1 path: GEMM tiles out[o,m], tanh, projection to u1 column ----
    d1_order = [0, 1, 2, 4, 5, 3]  # stream-arrival order (SP/Pool/ACT split)
    d1_ps = {}

    def emit_d1_tile(t):
        ps = gemm.tile([128, M], f32, tag="g1")
        for c in range(HC):
            pe_mm(
                ps, wd1_sb[:, t, c, :], dnsT_sb[:, c, :],
                start=(c == 0), stop=(c == HC - 1),
            )
        d1_ps[t] = ps

    def emit_d1_tail(t, i):
        th = tanhp.tile([128, M], bf16, tag="td1")
        nc.scalar.activation(th, d1_ps[t], Tanh)
        pe_mm(
            u1c, th, wcols_sb[:, t : t + 1], start=(i == 0), stop=(i == HC - 1)
        )

    emit_d1_tile(d1_order[0])
    for i in range(1, HC):
        emit_d1_tile(d1_order[i])
        emit_d1_tail(d1_order[i - 1], i - 1)
    emit_d1_tail(d1_order[HC - 1], HC - 1)

    e1c = work.tile([128, 1], bf16, tag="e1c")
    nc.scalar.activation(e1c, u1c, Exp)

    # ---- t2 path: GEMM tiles out[o,l]; tanh + projection per l-half so the
    # tail pipelines; d1 epilogue matmuls interleave into the PE stream ----
    t2_ps = {}

    def emit_t2_tile(t):
        ps = gemm.tile([128, L], f32, tag="g2")
        for c in range(HC):
            pe_mm(
                ps, wt2_sb[:, t, c, :], textT_sb[:, c, :],
                start=(c == 0), stop=(c == HC - 1),
            )
        t2_ps[t] = ps

    def emit_t2_tail(t):
        for j in range(2):
            th = tanhp.tile([128, 128], bf16, tag="tt2")
            nc.scalar.activation(th, t2_ps[t][:, j * 128 : (j + 1) * 128], Tanh)
            pe_mm(
                u2c[j], th, wcols_sb[:, HC + t : HC + t + 1],
                start=(t == 0), stop=(t == HC - 1),
            )

    emit_t2_tile(0)
    emit_t2_tile(1)
    emit_t2_tail(0)

    # d1 epilogue: s1 broadcast-sum + unnormalized v1, then one PSUM-source
    # DMA out.  (PE reaches these after t2 tile1, when e1c/dnsN are ready.)
    pe_mm(sv_ps[:, 0:1], ones_mat, e1c, start=True, stop=True)
    for c in range(HC):
        pe_mm(
            sv_ps[:, 1 + c : 2 + c], dnsN_sb[:, c * 128 : (c + 1) * 128], e1c,
            start=True, stop=True,
        )
    v1_sb = work.tile([128, 1 + HC], f32, tag="v1sb")
    nc.vector.tensor_copy(out=v1_sb, in_=sv_ps[:, 0 : 1 + HC])
    nc.sync.dma_start(out=v1_out, in_=v1_sb)

    emit_t2_tile(2)
    emit_t2_tail(1)
    emit_t2_tile(3)
    emit_t2_tail(2)
    emit_t2_tile(4)
    emit_t2_tail(3)
    emit_t2_tile(5)
    emit_t2_tail(4)
    emit_t2_tail(5)

    # t2 epilogue: exp per half, s2 broadcast-sum + unnormalized v2, DMA out.
    e2c0 = work.tile([128, 1], bf16, tag="e2c0")
    nc.scalar.activation(e2c0, u2c[0], Exp)
    e2c1 = work.tile([128, 1], bf16, tag="e2c1")
    nc.scalar.activation(e2c1, u2c[1], Exp)
    e2c = (e2c0, e2c1)
    pe_mm(sv_ps[:, 1 + HC : 2 + HC], ones_mat, e2c0, start=True, stop=False)
    pe_mm(sv_ps[:, 1 + HC : 2 + HC], ones_mat, e2c1, start=False, stop=True)
    for c in range(HC):
        for j in range(2):
            pe_mm(
                sv_ps[:, 2 + HC + c : 3 + HC + c],
                textN_sb[:, j, c * 128 : (c + 1) * 128], e2c[j],
                start=(j == 0), stop=(j == 1),
            )
    v2_sb = work.tile([128, 1 + HC], f32, tag="v2sb")
    nc.vector.tensor_copy(out=v2_sb, in_=sv_ps[:, 1 + HC : 2 + 2 * HC])
    nc.sync.dma_start(out=v2_out, in_=v2_sb)


_NC_CACHE = {}


def _get_module(reps=1):
    if reps not in _NC_CACHE:
        _NC_CACHE[reps] = _build_module(reps)
    return _NC_CACHE[reps]


def _bf16(x):
    return np.ascontiguousarray(np.asarray(x, np.float32).astype(ml_dtypes.bfloat16))


def _make_in_maps(kernel_inputs):
    text = np.asarray(kernel_inputs["text_features"], np.float32)
    dns = np.asarray(kernel_inputs["dns_features"], np.float32)
    W_d1 = np.asarray(kernel_inputs["W_d1"], np.float32)
    W_t2 = np.asarray(kernel_inputs["W_t2"], np.float32)
    wb = np.asarray(kernel_inputs["w_att1"], np.float32)[H:]
    wd = np.asarray(kernel_inputs["w_att2"], np.float32)[H:]

    # wf[p, t, c, o] = W[t*128+o, c*128+p]
    def wflip(W):
        return _bf16(
            W.reshape(HC, 128, HC, 128).transpose(3, 0, 2, 1).reshape(128, HC * HC * 128)
        )

    wd1fv = wflip(W_d1)
    wt2fv = wflip(W_t2)
    wcolsv = np.concatenate([wb.reshape(HC, 128).T, wd.reshape(HC, 128).T], axis=1)

    def chunkT(x, inner):  # [R, H] -> [128, HC*inner]; [p, c*inner+r] = x[r, c*128+p]
        return _bf16(
            x.T.reshape(HC, 128, inner).transpose(1, 0, 2).reshape(128, HC * inner)
        )

    in_maps = []
    for b in range(B):
        in_maps.append(
            {
                "dnsT": _bf16(
                    np.concatenate([chunkT(dns[b], M), wcolsv], axis=1)
                ),
                "textT": chunkT(text[b], L),
                "dnsN": _bf16(dns[b]),
                "textN": _bf16(
                    text[b].reshape(2, 128, H).transpose(1, 0, 2).reshape(128, 2 * H)
                ),
                "wd1f": wd1fv,
                "wt2f": wt2fv,
            }
        )
    return in_maps


def _run_device(kernel_inputs):
    from concourse.bass_utils import run_bass_kernel_spmd

    in_maps = _make_in_maps(kernel_inputs)
    nc = _get_module()
    return run_bass_kernel_spmd(nc, in_maps, list(range(B)))


def kernel(**inputs):
    res = _run_device(inputs)
    att_text = np.empty((B, L, H), np.float32)
    att_dns = np.empty((B, L, H), np.float32)
    for b in range(B):
        r = res.results[b]
        r1 = np.asarray(r["v1"], np.float32)  # [128, 1+HC]: sum | v chunks
        r2 = np.asarray(r["v2"], np.float32)
        v1 = (r1[:, 1:] / r1[:, 0:1]).T.reshape(H)
        v2 = (r2[:, 1:] / r2[:, 0:1]).T.reshape(H)
        att_dns[b] = v1[None, :]
        att_text[b] = v2[None, :]
    return att_text, att_dns
